# revision 31
# baseline (speedup 1.0000x reference)
"""Trainium2 Bass kernel for DensityGCNProcessor.

Model: 2-layer GCN over a per-sample kNN graph built from 1-D density values
(K=4 nearest by |density_i - density_j|), symmetric deg^-1/2 normalization on
target indegree, relu after each layer.

Strategy
--------
kNN in a 1-D metric means: after sorting nodes by density, every node's 4
nearest neighbours lie within +/-4 sorted positions, so the aggregation matrix
is a 9-diagonal banded matrix in sorted order. The host does the index math
(argsort, band weights w9 with exact reference tie-breaking), gathers node
features into sorted order and applies the layer-1 band (9 vectorized FMAs in
f32). The device runs the dense/compute-heavy 97% of FLOPs:

  1. L1 dense: H^T = relu(W1^T A1^T + b1)           (stationary W1, bf16)
  2. L2 dense: T2 window tiles = hT_win^T @ W2      (node-major, 8-row overlap)
  3. L2 agg:   out = relu(BandL2^T @ T2win + b2)    (single K=128 matmul/tile)
  4. contiguous bf16 DMA of the sorted-order output; host un-permutes/upcasts.

Work is tiled as 18 output tiles of 120 rows whose 128-row input windows
overlap by 8 rows, so the L2 band matmul is a single K=128 instruction —
no halo matmuls, no transposes, no gather/scatter DMA. Tolerance is 2e-2;
this lands ~3.8e-3.

Sharding: 8 cores = 4 batches x 2 rank-halves. Core c handles batch c//2,
sorted ranks [ (c%2)*2048, (c%2)*2048+2048 ).
"""

import numpy as np
import ml_dtypes

BF16 = ml_dtypes.bfloat16

# ---------------------------------------------------------------- constants
B = 4
CIN = 256
CHID = 512
COUT = 256
H = W = 64
N = H * W            # 4096 nodes per batch
KNN = 4
BAND = 4             # kNN lies within +/-4 sorted positions
HALF = N // 2        # 2048 ranks per core
TR = 120             # output rows per tile (window = TR + 2*BAND = 128)
NT = 18              # tiles: covers 2160 >= 2048 + 2*BAND halo rows
NA = NT * TR         # 2160 a1/h rows (valid: 2056)
NGA = NT * TR + 8    # 2168 gathered window rows
AW = 2176            # allocated hT free size (2160 + 16 pad)

_COMPILED = {}


# ---------------------------------------------------------------- host graph
def _build_band_weights(d_flat):
    """order [N], w9 [N, 9] f32: out_s[r] = sum_o w9[r, o+4] * g_s[r+o]."""
    order = np.argsort(d_flat, kind="stable")
    d_s = d_flat[order]

    offs = np.arange(-BAND, BAND + 1)
    ridx = np.arange(N)[:, None] + offs[None, :]
    valid = (ridx >= 0) & (ridx < N)
    ridx_c = np.clip(ridx, 0, N - 1)
    c = np.abs(d_s[ridx_c] - d_s[:, None]).astype(np.float32)
    c = np.where(valid, c, np.float32(np.inf))
    cand_j = np.where(valid, order[ridx_c], N)

    # reference = stable argsort over the full row: ties by smaller orig index.
    sel = np.lexsort((cand_j, c), axis=1)
    tgt_s = np.take_along_axis(ridx_c, sel[:, 1:KNN + 1], axis=1).reshape(-1)
    src_s = np.repeat(np.arange(N), KNN)

    deg = np.ones(N, dtype=np.float32)
    np.add.at(deg, tgt_s, np.float32(1.0))
    dinv = (np.float32(1.0) / np.sqrt(deg)).astype(np.float32)

    m = np.zeros((N, 9), dtype=np.float32)
    np.add.at(m, (tgt_s, src_s - tgt_s + BAND), np.float32(1.0))
    m[:, BAND] += 1.0  # self loops

    ro = np.arange(N)[:, None] + offs[None, :]
    rov = (ro >= 0) & (ro < N)
    w9 = m * dinv[:, None] * dinv[np.clip(ro, 0, N - 1)] * rov
    return order.astype(np.int32), w9.astype(np.float32)


def _host_graph(density_maps):
    """Per-core gather indices, L1 band rows, L2 band tiles. 8 dicts."""
    qq = np.arange(128)[:, None, None]            # window row within tile
    tt = np.arange(NT)[None, :, None]             # tile
    rr = np.arange(TR)[None, None, :]             # out row within tile
    col = qq - rr                                 # w9 column (o + 4)
    colv = (col >= 0) & (col <= 8)
    col_c = np.clip(col, 0, 8)
    ii = TR * tt + rr                             # flat out-row index

    per_core = []
    for b in range(B):
        d = np.asarray(density_maps[b]).reshape(N).astype(np.float32)
        order, w9 = _build_band_weights(d)
        w9x = np.concatenate([w9, np.zeros((1, 9), np.float32)])  # row N = 0
        for half in range(2):
            r0 = half * HALF

            # gather source: window row j -> orig node (rank r0 - 8 + j)
            jr = r0 - 8 + np.arange(NGA)
            okj = (jr >= 0) & (jr < N)
            src = np.where(okj, order[np.clip(jr, 0, N - 1)], 0)

            # L1 band rows: out row i -> rank g1 = r0 - 4 + i (valid i < 2056)
            iflat = np.arange(NA)
            g1 = r0 - 4 + iflat
            ok1 = (g1 >= 0) & (g1 < N) & (iflat < HALF + 2 * BAND)
            wrows = w9x[np.where(ok1, g1, N)]     # [NA, 9]

            # L2 band tiles: out row i -> rank g2 = r0 + i (valid i < 2048)
            gi2 = np.where(ii < HALF, r0 + ii, N)
            bl2 = w9x[np.broadcast_to(gi2, (128, NT, TR)),
                      np.broadcast_to(col_c, (128, NT, TR))] * colv

            per_core.append(dict(order=order, src=src, wrows=wrows,
                                 bl2=bl2.astype(BF16)))
    return per_core


# ---------------------------------------------------------------- device IR
def build_nc():
    import concourse.bass as bass
    import concourse.bacc as bacc
    import concourse.mybir as mybir
    from concourse.tile import TileContext

    F32 = mybir.dt.float32
    BF = mybir.dt.bfloat16

    nc = bacc.Bacc()
    a1t = nc.dram_tensor("a1t", [128, 2, NA], BF, kind="ExternalInput")
    bl2 = nc.dram_tensor("bl2", [128, NT, TR], BF, kind="ExternalInput")
    w1 = nc.dram_tensor("w1", [128, 2, CHID], BF, kind="ExternalInput")
    w2 = nc.dram_tensor("w2", [128, 4, COUT], BF, kind="ExternalInput")
    b1 = nc.dram_tensor("b1", [128, 4], F32, kind="ExternalInput")
    b2rep = nc.dram_tensor("b2rep", [128, COUT], F32, kind="ExternalInput")
    out_d = nc.dram_tensor("out_d", [NA, COUT], BF, kind="ExternalOutput")

    BW = [128, 512, 512, 512, NA - 1664]   # a1T block widths (small head
                                           # block so the PE starts early)
    OCHS = [3, 3, 3, 3, 3, 2, 1]           # out DMA chunks (small tail)
    RELU = mybir.ActivationFunctionType.Relu
    COPY = mybir.ActivationFunctionType.Copy

    with TileContext(nc) as tc:
        with (
            tc.tile_pool(name="const", bufs=1) as cpool,
            tc.tile_pool(name="stream", bufs=3) as sp,
            tc.tile_pool(name="psum", bufs=2, space="PSUM") as pp,
            tc.tile_pool(name="psumd", bufs=3, space="PSUM") as pd,
        ):
            # a1T blocks stream in on sync; each dense block starts as soon as
            # its chunk lands. Weights ride the scalar queue in parallel;
            # later-needed tensors queue behind a1T on sync.
            a1Tb = []
            o = 0
            for b_ in range(5):
                at_ = cpool.tile([128, 2, BW[b_]], BF, tag=f"a1b{b_}")
                nc.sync.dma_start(at_, a1t[:, :, o:o + BW[b_]])
                a1Tb.append(at_)
                o += BW[b_]
            w1_sb = cpool.tile([128, 2, CHID], BF)
            nc.scalar.dma_start(w1_sb, w1[:, :, :])
            b1_sb = cpool.tile([128, 4], F32)
            nc.scalar.dma_start(b1_sb, b1[:, :])
            w2_sb = cpool.tile([128, 4, COUT], BF)
            nc.scalar.dma_start(w2_sb, w2[:, :, :])
            bl2_sb = cpool.tile([128, NT, TR], BF)
            nc.sync.dma_start(bl2_sb, bl2[:, :, :])
            b2_sb = cpool.tile([128, COUT], F32)
            nc.sync.dma_start(b2_sb, b2rep[:, :])

            hT = cpool.tile([128, 4, AW], BF)
            # pad cols [NA, AW) must be finite: tile 17's lhsT window reads them
            nc.gpsimd.memset(hT[:, :, NA:AW], 0.0)

            # ---------------- L1 dense: H^T = relu(W1^T A1^T + b1) (chid-major)
            rl = 0
            lo = 0
            for b_ in range(5):
                w_ = BW[b_]
                for mb in range(4):
                    psH = pd.tile([128, 512], F32, tag="d1", space="PSUM")
                    for kb in range(2):
                        nc.tensor.matmul(
                            psH[:, 0:w_],
                            lhsT=w1_sb[:, kb, 128 * mb:128 * (mb + 1)],
                            rhs=a1Tb[b_][:, kb, 0:w_],
                            start=(kb == 0), stop=(kb == 1))
                    dst = hT[:, mb, lo:lo + w_]
                    src = psH[:, 0:w_]
                    if rl % 2 == 0:
                        nc.scalar.activation(dst, src, RELU,
                                             bias=b1_sb[:, mb:mb + 1],
                                             scale=1.0)
                    else:
                        nc.vector.tensor_scalar(
                            dst, src,
                            scalar1=b1_sb[:, mb:mb + 1], scalar2=0.0,
                            op0=mybir.AluOpType.add, op1=mybir.AluOpType.max)
                    rl += 1
                lo += w_

            # ---------------- L2 dense (node-major window tiles) + L2 agg
            out_sb, out_of = [], []
            o = 0
            for k, ch in enumerate(OCHS):
                os_t = cpool.tile([128, ch, COUT], BF, tag=f"os{k}")
                out_sb.append(os_t)
                out_of.append(o)
                o += ch
            OENG = [nc.gpsimd, nc.sync, nc.gpsimd, nc.sync, nc.gpsimd,
                    nc.sync, nc.scalar]
            ob = 0  # current out chunk
            for t in range(NT):
                psT = pp.tile([128, COUT], F32, tag="d2", space="PSUM")
                for kb in range(4):
                    nc.tensor.matmul(
                        psT,
                        lhsT=hT[:, kb, TR * t:TR * t + 128],
                        rhs=w2_sb[:, kb, :],
                        start=(kb == 0), stop=(kb == 3))
                t2w = sp.tile([128, COUT], BF, tag="t2w")
                if t % 2 == 0:
                    nc.scalar.activation(t2w, psT, COPY)
                else:
                    nc.vector.tensor_copy(t2w, psT)

                psO = pp.tile([TR, COUT], F32, tag="agO", space="PSUM")
                nc.tensor.matmul(psO, lhsT=bl2_sb[:, t, :], rhs=t2w,
                                 start=True, stop=True)
                dst = out_sb[ob][0:TR, t - out_of[ob], :]
                nc.vector.tensor_tensor(out=dst, in0=psO, in1=b2_sb[0:TR, :],
                                        op=mybir.AluOpType.add)
                nc.scalar.activation(dst, dst, RELU)
                if t - out_of[ob] == OCHS[ob] - 1:
                    lo = TR * out_of[ob]
                    hi = TR * (out_of[ob] + OCHS[ob])
                    OENG[ob].dma_start(
                        out_d[lo:hi, :].rearrange("(t p) c -> p t c", p=TR),
                        out_sb[ob][0:TR, :, :])
                    ob += 1

    nc.compile()
    return nc


def make_in_maps(density_maps, feature_maps, W1, b1, W2, b2):
    graph = _host_graph(density_maps)
    fm = np.ascontiguousarray(np.asarray(feature_maps, dtype=np.float32))
    w1p = np.asarray(W1, np.float32).reshape(2, 128, CHID) \
        .transpose(1, 0, 2).astype(BF16)
    w2p = np.asarray(W2, np.float32).reshape(4, 128, COUT) \
        .transpose(1, 0, 2).astype(BF16)
    b1p = np.ascontiguousarray(np.asarray(b1, np.float32).reshape(4, 128).T)
    b2r = np.broadcast_to(np.asarray(b2, np.float32), (128, COUT)).copy()

    in_maps = []
    for c in range(8):
        g = graph[c]
        xs = fm[c // 2].reshape(CIN, N).T[g["src"]]      # [NGA, CIN] f32
        # layer-1 band aggregation in f32 on host: 9 shifted FMAs
        wr = g["wrows"]                                  # [NA, 9]
        a1 = wr[:, 0:1] * xs[0:NA]
        for o in range(1, 9):
            a1 += wr[:, o:o + 1] * xs[o:o + NA]
        a1tp = np.ascontiguousarray(
            a1.T.reshape(2, 128, NA).transpose(1, 0, 2)).astype(BF16)
        in_maps.append({
            "a1t": a1tp, "bl2": np.ascontiguousarray(g["bl2"]),
            "w1": w1p, "w2": w2p, "b1": b1p, "b2rep": b2r,
        })
    return in_maps, graph


def kernel(density_maps, feature_maps, W1, b1, W2, b2):
    from concourse.bass_utils import run_bass_kernel_spmd

    if "nc" not in _COMPILED:
        _COMPILED["nc"] = build_nc()
    nc = _COMPILED["nc"]

    in_maps, graph = make_in_maps(density_maps, feature_maps, W1, b1, W2, b2)
    res = run_bass_kernel_spmd(nc, in_maps, core_ids=list(range(8)))

    out = np.empty((B, N, COUT), dtype=np.float32)
    for b in range(B):
        o0 = res.results[2 * b]["out_d"][:HALF].astype(np.float32)
        o1 = res.results[2 * b + 1]["out_d"][:HALF].astype(np.float32)
        out[b][graph[2 * b]["order"]] = np.concatenate([o0, o1], axis=0)
    return np.ascontiguousarray(
        out.reshape(B, H, W, COUT).transpose(0, 3, 1, 2)).astype(np.float32)


# revision 35
# speedup vs baseline: 1.0310x; 1.0310x over previous
"""Trainium2 Bass kernel for DensityGCNProcessor.

Model: 2-layer GCN over a per-sample kNN graph built from 1-D density values
(K=4 nearest by |density_i - density_j|), symmetric deg^-1/2 normalization on
target indegree, relu after each layer.

Strategy
--------
kNN in a 1-D metric means: after sorting nodes by density, every node's 4
nearest neighbours lie within +/-4 sorted positions, so the aggregation matrix
is a 9-diagonal banded matrix in sorted order. The host does the index math
(argsort, band weights w9 with exact reference tie-breaking), gathers node
features into sorted order and applies the layer-1 band (9 vectorized FMAs in
f32). The device runs the dense/compute-heavy 97% of FLOPs:

  1. L1 dense: H^T = relu(W1^T A1^T + b1)           (stationary W1, bf16)
  2. L2 dense: T2 window tiles = hT_win^T @ W2      (node-major, 8-row overlap)
  3. L2 agg:   out = relu(BandL2^T @ T2win + b2)    (single K=128 matmul/tile)
  4. contiguous bf16 DMA of the sorted-order output; host un-permutes/upcasts.

Work is tiled as 18 output tiles of 120 rows whose 128-row input windows
overlap by 8 rows, so the L2 band matmul is a single K=128 instruction —
no halo matmuls, no transposes, no gather/scatter DMA. Tolerance is 2e-2;
this lands ~3.8e-3.

Sharding: 8 cores = 4 batches x 2 rank-halves. Core c handles batch c//2,
sorted ranks [ (c%2)*2048, (c%2)*2048+2048 ).
"""

import numpy as np
import ml_dtypes

BF16 = ml_dtypes.bfloat16

# ---------------------------------------------------------------- constants
B = 4
CIN = 256
CHID = 512
COUT = 256
H = W = 64
N = H * W            # 4096 nodes per batch
KNN = 4
BAND = 4             # kNN lies within +/-4 sorted positions
HALF = N // 2        # 2048 ranks per core
TR = 120             # output rows per tile (window = TR + 2*BAND = 128)
NT = 18              # tiles: covers 2160 >= 2048 + 2*BAND halo rows
NA = NT * TR         # 2160 a1/h rows (valid: 2056)
NGA = NT * TR + 8    # 2168 gathered window rows
AW = 2176            # allocated hT free size (2160 + 16 pad)

_COMPILED = {}


# ---------------------------------------------------------------- host graph
def _build_band_weights(d_flat):
    """order [N], w9 [N, 9] f32: out_s[r] = sum_o w9[r, o+4] * g_s[r+o]."""
    order = np.argsort(d_flat, kind="stable")
    d_s = d_flat[order]

    offs = np.arange(-BAND, BAND + 1)
    ridx = np.arange(N)[:, None] + offs[None, :]
    valid = (ridx >= 0) & (ridx < N)
    ridx_c = np.clip(ridx, 0, N - 1)
    c = np.abs(d_s[ridx_c] - d_s[:, None]).astype(np.float32)
    c = np.where(valid, c, np.float32(np.inf))
    cand_j = np.where(valid, order[ridx_c], N)

    # reference = stable argsort over the full row: ties by smaller orig index.
    sel = np.lexsort((cand_j, c), axis=1)
    tgt_s = np.take_along_axis(ridx_c, sel[:, 1:KNN + 1], axis=1).reshape(-1)
    src_s = np.repeat(np.arange(N), KNN)

    deg = np.ones(N, dtype=np.float32)
    np.add.at(deg, tgt_s, np.float32(1.0))
    dinv = (np.float32(1.0) / np.sqrt(deg)).astype(np.float32)

    m = np.zeros((N, 9), dtype=np.float32)
    np.add.at(m, (tgt_s, src_s - tgt_s + BAND), np.float32(1.0))
    m[:, BAND] += 1.0  # self loops

    ro = np.arange(N)[:, None] + offs[None, :]
    rov = (ro >= 0) & (ro < N)
    w9 = m * dinv[:, None] * dinv[np.clip(ro, 0, N - 1)] * rov
    return order.astype(np.int32), w9.astype(np.float32)


def _host_graph(density_maps):
    """Per-core gather indices, L1 band rows, L2 band tiles. 8 dicts."""
    qq = np.arange(128)[:, None, None]            # window row within tile
    tt = np.arange(NT)[None, :, None]             # tile
    rr = np.arange(TR)[None, None, :]             # out row within tile
    col = qq - rr                                 # w9 column (o + 4)
    colv = (col >= 0) & (col <= 8)
    col_c = np.clip(col, 0, 8)
    ii = TR * tt + rr                             # flat out-row index

    per_core = []
    for b in range(B):
        d = np.asarray(density_maps[b]).reshape(N).astype(np.float32)
        order, w9 = _build_band_weights(d)
        w9x = np.concatenate([w9, np.zeros((1, 9), np.float32)])  # row N = 0
        for half in range(2):
            r0 = half * HALF

            # gather source: window row j -> orig node (rank r0 - 8 + j)
            jr = r0 - 8 + np.arange(NGA)
            okj = (jr >= 0) & (jr < N)
            src = np.where(okj, order[np.clip(jr, 0, N - 1)], 0)

            # L1 band rows: out row i -> rank g1 = r0 - 4 + i (valid i < 2056)
            iflat = np.arange(NA)
            g1 = r0 - 4 + iflat
            ok1 = (g1 >= 0) & (g1 < N) & (iflat < HALF + 2 * BAND)
            wrows = w9x[np.where(ok1, g1, N)]     # [NA, 9]

            # L2 band tiles: out row i -> rank g2 = r0 + i (valid i < 2048)
            gi2 = np.where(ii < HALF, r0 + ii, N)
            bl2 = w9x[np.broadcast_to(gi2, (128, NT, TR)),
                      np.broadcast_to(col_c, (128, NT, TR))] * colv

            per_core.append(dict(order=order, src=src, wrows=wrows,
                                 bl2=bl2.astype(BF16)))
    return per_core


# ---------------------------------------------------------------- device IR
def build_nc(use_b2):
    import concourse.bass as bass
    import concourse.bacc as bacc
    import concourse.mybir as mybir
    from concourse.tile import TileContext

    F32 = mybir.dt.float32
    BF = mybir.dt.bfloat16

    nc = bacc.Bacc()
    a1t = nc.dram_tensor("a1t", [128, 2, NA], BF, kind="ExternalInput")
    bl2 = nc.dram_tensor("bl2", [128, NT, TR], BF, kind="ExternalInput")
    w1 = nc.dram_tensor("w1", [128, 2, CHID], BF, kind="ExternalInput")
    w2 = nc.dram_tensor("w2", [128, 4, COUT], BF, kind="ExternalInput")
    b1 = nc.dram_tensor("b1", [128, 4], F32, kind="ExternalInput")
    b2rep = nc.dram_tensor("b2rep", [128, COUT], F32, kind="ExternalInput")
    out_d = nc.dram_tensor("out_d", [NA, COUT], BF, kind="ExternalOutput")

    BW = [128, 512, 512, 512, NA - 1664]   # a1T block widths (small head
                                           # block so the PE starts early)
    OCHS = [3, 3, 3, 3, 3, 2, 1]           # out DMA chunks (small tail)
    RELU = mybir.ActivationFunctionType.Relu
    COPY = mybir.ActivationFunctionType.Copy

    with TileContext(nc) as tc:
        with (
            tc.tile_pool(name="const", bufs=1) as cpool,
            tc.tile_pool(name="stream", bufs=3) as sp,
            tc.tile_pool(name="psum", bufs=2, space="PSUM") as pp,
            tc.tile_pool(name="psumd", bufs=3, space="PSUM") as pd,
        ):
            # a1T blocks stream in on sync; each dense block starts as soon as
            # its chunk lands. Weights ride the scalar queue in parallel;
            # later-needed tensors queue behind a1T on sync.
            a1Tb = []
            o = 0
            for b_ in range(5):
                at_ = cpool.tile([128, 2, BW[b_]], BF, tag=f"a1b{b_}")
                nc.sync.dma_start(at_, a1t[:, :, o:o + BW[b_]])
                a1Tb.append(at_)
                o += BW[b_]
            w1_sb = cpool.tile([128, 2, CHID], BF)
            nc.scalar.dma_start(w1_sb, w1[:, :, :])
            b1_sb = cpool.tile([128, 4], F32)
            nc.scalar.dma_start(b1_sb, b1[:, :])
            w2_sb = cpool.tile([128, 4, COUT], BF)
            nc.scalar.dma_start(w2_sb, w2[:, :, :])
            bl2_sb = cpool.tile([128, NT, TR], BF)
            nc.sync.dma_start(bl2_sb, bl2[:, :, :])
            if use_b2:
                b2_sb = cpool.tile([128, COUT], F32)
                nc.sync.dma_start(b2_sb, b2rep[:, :])

            hT = cpool.tile([128, 4, AW], BF)
            # pad cols [NA, AW) must be finite: tile 17's lhsT window reads them
            nc.gpsimd.memset(hT[:, :, NA:AW], 0.0)

            # ---------------- L1 dense: H^T = relu(W1^T A1^T + b1) (chid-major)
            rl = 0
            lo = 0
            for b_ in range(5):
                w_ = BW[b_]
                for mb in range(4):
                    psH = pd.tile([128, 512], F32, tag="d1", space="PSUM")
                    for kb in range(2):
                        nc.tensor.matmul(
                            psH[:, 0:w_],
                            lhsT=w1_sb[:, kb, 128 * mb:128 * (mb + 1)],
                            rhs=a1Tb[b_][:, kb, 0:w_],
                            start=(kb == 0), stop=(kb == 1))
                    dst = hT[:, mb, lo:lo + w_]
                    src = psH[:, 0:w_]
                    if rl % 2 == 0:
                        nc.scalar.activation(dst, src, RELU,
                                             bias=b1_sb[:, mb:mb + 1],
                                             scale=1.0)
                    else:
                        nc.vector.tensor_scalar(
                            dst, src,
                            scalar1=b1_sb[:, mb:mb + 1], scalar2=0.0,
                            op0=mybir.AluOpType.add, op1=mybir.AluOpType.max)
                    rl += 1
                lo += w_

            # ---------------- L2 dense (node-major window tiles) + L2 agg
            out_sb, out_of = [], []
            o = 0
            for k, ch in enumerate(OCHS):
                os_t = cpool.tile([128, ch, COUT], BF, tag=f"os{k}")
                out_sb.append(os_t)
                out_of.append(o)
                o += ch
            OENG = [nc.gpsimd, nc.sync, nc.gpsimd, nc.sync, nc.gpsimd,
                    nc.sync, nc.scalar]
            ob = 0  # current out chunk
            for t in range(NT):
                psT = pp.tile([128, COUT], F32, tag="d2", space="PSUM")
                for kb in range(4):
                    nc.tensor.matmul(
                        psT,
                        lhsT=hT[:, kb, TR * t:TR * t + 128],
                        rhs=w2_sb[:, kb, :],
                        start=(kb == 0), stop=(kb == 3))
                t2w = sp.tile([128, COUT], BF, tag="t2w")
                if t % 2 == 0:
                    nc.scalar.activation(t2w, psT, COPY)
                else:
                    nc.vector.tensor_copy(t2w, psT)

                psO = pp.tile([TR, COUT], F32, tag="agO", space="PSUM")
                nc.tensor.matmul(psO, lhsT=bl2_sb[:, t, :], rhs=t2w,
                                 start=True, stop=True)
                dst = out_sb[ob][0:TR, t - out_of[ob], :]
                if use_b2:
                    nc.vector.tensor_tensor(out=dst, in0=psO,
                                            in1=b2_sb[0:TR, :],
                                            op=mybir.AluOpType.add)
                    nc.scalar.activation(dst, dst, RELU)
                elif t % 2 == 0:
                    nc.vector.tensor_scalar(dst, psO, scalar1=0.0,
                                            scalar2=None,
                                            op0=mybir.AluOpType.max)
                else:
                    nc.scalar.activation(dst, psO, RELU)
                if t - out_of[ob] == OCHS[ob] - 1:
                    lo = TR * out_of[ob]
                    hi = TR * (out_of[ob] + OCHS[ob])
                    OENG[ob].dma_start(
                        out_d[lo:hi, :].rearrange("(t p) c -> p t c", p=TR),
                        out_sb[ob][0:TR, :, :])
                    ob += 1

    nc.compile()
    return nc


def make_in_maps(density_maps, feature_maps, W1, b1, W2, b2):
    graph = _host_graph(density_maps)
    fm = np.ascontiguousarray(np.asarray(feature_maps, dtype=np.float32))
    w1p = np.asarray(W1, np.float32).reshape(2, 128, CHID) \
        .transpose(1, 0, 2).astype(BF16)
    w2p = np.asarray(W2, np.float32).reshape(4, 128, COUT) \
        .transpose(1, 0, 2).astype(BF16)
    b1p = np.ascontiguousarray(np.asarray(b1, np.float32).reshape(4, 128).T)
    b2r = np.broadcast_to(np.asarray(b2, np.float32), (128, COUT)).copy()

    in_maps = []
    for c in range(8):
        g = graph[c]
        xs = fm[c // 2].reshape(CIN, N).T[g["src"]]      # [NGA, CIN] f32
        # layer-1 band aggregation in f32 on host: 9 shifted FMAs
        wr = g["wrows"]                                  # [NA, 9]
        a1 = wr[:, 0:1] * xs[0:NA]
        for o in range(1, 9):
            a1 += wr[:, o:o + 1] * xs[o:o + NA]
        a1tp = np.ascontiguousarray(
            a1.T.reshape(2, 128, NA).transpose(1, 0, 2)).astype(BF16)
        in_maps.append({
            "a1t": a1tp, "bl2": np.ascontiguousarray(g["bl2"]),
            "w1": w1p, "w2": w2p, "b1": b1p, "b2rep": b2r,
        })
    return in_maps, graph


def kernel(density_maps, feature_maps, W1, b1, W2, b2):
    from concourse.bass_utils import run_bass_kernel_spmd

    use_b2 = bool(np.any(np.asarray(b2)))
    if use_b2 not in _COMPILED:
        _COMPILED[use_b2] = build_nc(use_b2)
    nc = _COMPILED[use_b2]

    in_maps, graph = make_in_maps(density_maps, feature_maps, W1, b1, W2, b2)
    res = run_bass_kernel_spmd(nc, in_maps, core_ids=list(range(8)))

    out = np.empty((B, N, COUT), dtype=np.float32)
    for b in range(B):
        o0 = res.results[2 * b]["out_d"][:HALF].astype(np.float32)
        o1 = res.results[2 * b + 1]["out_d"][:HALF].astype(np.float32)
        out[b][graph[2 * b]["order"]] = np.concatenate([o0, o1], axis=0)
    return np.ascontiguousarray(
        out.reshape(B, H, W, COUT).transpose(0, 3, 1, 2)).astype(np.float32)
